# revision 36
# baseline (speedup 1.0000x reference)
"""Trainium2 Bass kernel for nn_AttentionEdgeDecoder.

Reference computation (per batch b):
  hn = h[b,:4096,:], hg = h[b,4096,:]
  q = hg @ W_q  (single query, 8 heads x 16 dims)
  k,v = hn @ W_kv ; attn = softmax(q.k/sqrt(16)) ; y = attn.v
  mh = y @ W_mhc ; y2[i] = <mh, hn[i]>             (4096 scalars)
  e[i,j] = y2[j]*W_lin[0,0] + y2[i]*W_lin[1,0]     (4096x4096 output)

Output is 4*4096^2*4B = 268MB -> HBM-write bound. Per-core the 33.5MB
write stream runs at ~427 GB/s (the DMA cap) and takes ~78us, so total
time = (time to first output byte) + 78us: the whole game is collapsing
the prologue. Sharding: 8 cores = 4 batches x 2 row-halves.

Trace-derived hardware facts this layout is built around:
 - HWDGE descriptor cost is ~fixed per descriptor, so DMA throughput is
   proportional to the per-partition line size; and the DMA engines drain
   the two rings in an unpredictable order when both have queued work, so
   ALL inputs go on the scalar ring in strict FIFO order (~305 GB/s with
   8.2KB lines) and the sync ring is output-only. hnT and hnp are shipped
   as ONE tensor in two column groups so the group-1 attention chain runs
   while group 2 is still streaming (score/exp/u tiles are split per
   group because dependency tracking is per-tile for multi-writer tiles,
   and a tile_wait_until hint stops the scheduler from hoisting the
   group-2 scores ahead of group-1's u matmuls).
 - DVE reading PSUM is ~1.6x slower than SBUF (1.1 vs 0.69 ns/elem), so
   most of the 16-tile e stream reads R from SBUF; the first NSPLIT
   row-tiles (latency-critical) read R straight from PSUM while the
   copies + PSUM-pool-close DRAIN finish in their shadow.
 - The PE p-state ramp needs ~3-4us of continuous high-utilization work;
   a small burst of bf16 warm-up matmuls on a memset tile (no DRAM
   dependency) overlaps the preamble. Mid-stream keep-hot bursts do NOT
   work: the tile scheduler hoists independent matmuls ahead of
   data-gated ones, delaying the real pipeline.

Per-core row ownership is made SPMD-uniform by ROTATING each core's node
axis so its own 2048 rows come first (host rolls hn by half*2048; the
host un-rotates the output columns after the run). This removes the hrT
input entirely: the col term's lhsT tiles are just hnT[:, t*128:...].

TensorEngine formulation (out = lhsT.T @ rhs), heavy ops bf16 1-pass:
  q_col   = matmul(lhsT=W_q, rhs=hg_col)                  [128,1]
  Qh      = headmask * q_col   (block-diag scatter)       [128,8]
  Wqeff   = matmul(lhsT=WkT, rhs=Qh) = Wk @ Qh            [128,8]
  sT      = matmul(lhsT=hnT_chunk, rhs=Wqeff)             [4096,8]
  pT      = exp(0.25*sT)      (no max-subtract: |s/4| < 8)
  u_ext   = sum_chunks matmul(lhsT=pT_chunk, rhs=hnp_chunk) [8,129]
            (hnp carries a ones column -> u_ext[:,128] = softmax denom)
  u       = u_ext[:, :128] * (1/ssum)  (one vector op PSUM->SBUF)
  uT      = transpose(u)                                  [128,8]
  ymatT   = matmul(lhsT=Wv, rhs=uT)                       [128,8]
  y_col   = reduce_h(ymatT * headmask)                    [128,1]
  y01     = y_col * [W0, W1]  (one vector op)             [128,2]
  mh01    = matmul(lhsT=W_mhc, rhs=y01)                   [128,2]
            -> mh0 (f32 col), mh1 (bf16 col); no K=1 broadcast matmuls
  mh0rep  = ones * mh0  (vector bcast along free, bf16)   [128,128]
  col     = matmul(lhsT=hnT_tile_t, rhs=mh1) = W1*y2[rows] [128,16]
  R       = matmul(lhsT=mh0rep, rhs=hnT) in PSUM          [128,4096]
            -> copied to SBUF r_sb chunk-by-chunk (scalar engine)
  e_tile  = tensor_scalar_add(R, col[:,t]) -> 2MB DMAs out (sync ring)

Tile 0 streams out in progressive pieces (512/512/1024/2048 cols) so
the first DMA fires right after R's first bank; group 2 of the input is
two DMAs (hnT then hnp) so the group-2 scores start before its hnp half
has landed.

Best measured: 110.4us (baseline at session start: 123.8us). Note the
device is bimodally noisy run-to-run (~110 vs ~129 for identical NEFFs,
apparently HBM contention external to this kernel).
"""

from contextlib import ExitStack

import ml_dtypes
import numpy as np

import concourse.bass as bass
import concourse.mybir as mybir
from concourse import bacc, tile
from concourse.bass_utils import run_bass_kernel_spmd

BP = 4
N = 4096
HID = 128
H = 8
D = 16
ROWS = N // 2          # 2048 rows per core
NT = ROWS // 128       # 16 row tiles per core
NJC = N // 128         # 32 node chunks
HP = HID + 1           # hnp chunk width (extra ones column -> softmax denom)
F32 = mybir.dt.float32
BF16 = mybir.dt.bfloat16

NWARM = 6              # PE warm-up matmuls (bf16 [128,512]) for p-state ramp
NWARM2 = 0             # (the tile scheduler hoists independent warm matmuls
NWARM3 = 0             #  ahead of data-gated work, so mid-stream keep-hot
NWARM4 = 0             #  bursts only delay the real pipeline; keep them off)
NSPLIT = 4             # first NSPLIT row tiles stream as half-width DMAs

# hna column groups: [hnT chunks 0-15 | hnp chunks 0-15 | hnT 16-31 | hnp 16-31]
# so that scores/u for the first 16 node chunks can start when the first
# group-pair of DMAs lands, pipelining compute against the input stream.
G1T = 0
G1P = N // 2                   # 2048
G2T = G1P + 16 * HP            # 4112
G2P = G2T + N // 2             # 6160
HNA_COLS = G2P + 16 * HP       # 8224


def _hnT_chunk(hna, jc, w=128):
    """AP of hnT columns [jc*w, (jc+1)*w) in the grouped layout (w | 2048)."""
    j0 = jc * w
    base = j0 if j0 < N // 2 else G2T + (j0 - N // 2)
    return hna[:, base : base + w]


def _hnp_chunk(hna, jc):
    base = (G1P + jc * HP) if jc < 16 else (G2P + (jc - 16) * HP)
    return hna[:, base : base + HP]

# wb column layout (bf16)
WKT0 = 0               # WkT = W_kv[:, :128].T
WV0 = HID              # Wv  = W_kv[:, 128:]
WMHC0 = 2 * HID
WQ0 = 3 * HID
HGB0 = 4 * HID         # hg column
ID0 = HGB0 + 1         # 8x8 identity (partitions 0..7)
MSK0 = ID0 + 8         # head mask [128, 8]
WL0 = MSK0 + 8         # wl_rep[:, k] = W_lin[k, 0] on every partition
WB_COLS = WL0 + 2


def build_bass():
    nc = bacc.Bacc()

    wb_ext = nc.declare_dram_parameter("wb", [HID, WB_COLS], BF16, isOutput=False)
    hna_ext = nc.declare_dram_parameter("hna", [128, HNA_COLS], BF16, isOutput=False)
    out_ext = nc.declare_dram_parameter("out", [ROWS, N], F32, isOutput=True)

    with tile.TileContext(nc) as tc, ExitStack() as ctx:
        sb = ctx.enter_context(tc.tile_pool(name="sb", bufs=1))
        small = ctx.enter_context(tc.tile_pool(name="small", bufs=1))
        epool = ctx.enter_context(tc.tile_pool(name="epool", bufs=8))

        # ---- input DMAs: 8.2KB lines, split across rings by partition and
        # into two column groups so compute pipelines against arrival.
        wb_sb = sb.tile([HID, WB_COLS], BF16)
        hna_sb = sb.tile([128, HNA_COLS], BF16)
        # all inputs on the scalar ring in strict FIFO order (the DMA
        # engines drain rings in an unpredictable order when both have
        # queued work, so a single ring is the only way to guarantee
        # group 1 lands before group 2); the sync ring is output-only.
        nc.scalar.dma_start(wb_sb[:], wb_ext[:, :])
        nc.scalar.dma_start(hna_sb[:, :G2T], hna_ext[:, :G2T])
        nc.scalar.dma_start(hna_sb[:, G2T:G2P], hna_ext[:, G2T:G2P])
        nc.scalar.dma_start(hna_sb[:, G2P:], hna_ext[:, G2P:])

        id8 = wb_sb[:, ID0 : ID0 + 8]
        mask_ap = wb_sb[:, MSK0 : MSK0 + H]
        wl_rep = wb_sb[:, WL0 : WL0 + 2]

        r_sb = sb.tile([128, N], F32)
        col_sb = small.tile([128, NT], F32)

        with ExitStack() as pctx:
            ps_small = pctx.enter_context(
                tc.tile_pool(name="ps_small", bufs=1, space="PSUM")
            )
            ps_tmp = pctx.enter_context(
                tc.tile_pool(name="ps_tmp", bufs=2, space="PSUM")
            )
            ps_warm = pctx.enter_context(
                tc.tile_pool(name="ps_warm", bufs=1, space="PSUM")
            )

            # ---- PE warm-up: no DRAM dependency, ramps the p-state clock
            warm_sb = small.tile([128, 512], BF16)
            nc.vector.memset(warm_sb[:], 1.0)
            ones_sb = small.tile([128, HID], BF16)
            nc.vector.memset(ones_sb[:], 1.0)
            for _ in range(NWARM):
                warm_ps = ps_warm.tile([128, 512], F32, tag="warm")
                nc.tensor.matmul(
                    warm_ps[:], warm_sb[:, :128], warm_sb[:], start=True, stop=True
                )

            # ---- attention prologue ----
            q_ps = ps_tmp.tile([HID, 1], F32, tag="tmp")
            nc.tensor.matmul(
                q_ps[:],
                wb_sb[:, WQ0 : WQ0 + HID],
                wb_sb[:, HGB0 : HGB0 + 1],
                start=True,
                stop=True,
            )
            q_sb = small.tile([HID, 1], F32)
            nc.scalar.copy(q_sb[:], q_ps[:])

            qh_sb = small.tile([HID, H], BF16)
            nc.vector.tensor_scalar_mul(qh_sb[:], mask_ap, q_sb[:])

            wq_ps = ps_tmp.tile([HID, H], F32, tag="tmp")
            nc.tensor.matmul(
                wq_ps[:], wb_sb[:, WKT0 : WKT0 + HID], qh_sb[:], start=True, stop=True
            )
            wqeff_sb = small.tile([HID, H], BF16)
            nc.scalar.copy(wqeff_sb[:], wq_ps[:])

            def warm(n):
                for _ in range(n):
                    wps = ps_warm.tile([128, 512], F32, tag="warm")
                    nc.tensor.matmul(
                        wps[:], warm_sb[:, :128], warm_sb[:], start=True, stop=True
                    )

            # keep the PE busy until input group 1 lands
            warm(NWARM2)

            # scores + exp + u per input column-group (separate tiles so the
            # group-1 chain runs while group 2 is still streaming in; dep
            # tracking is per-tile for multi-writer tiles). Keep-hot warm
            # matmuls fill the PE gaps so the p-state clock stays up.
            pT_g = []
            u_g = []
            for g in range(2):
                # tell the scheduler group-2 data really lands late, so it
                # orders group-1's u matmuls ahead of the group-2 scores
                wctx = tc.tile_wait_until(0.016, enable=(g == 1))
                wctx.__enter__()
                sT_ps = ps_small.tile([128, NJC // 2, H], F32, tag=f"sT{g}")
                for i in range(NJC // 2):
                    nc.tensor.matmul(
                        sT_ps[:, i, :],
                        _hnT_chunk(hna_sb, g * (NJC // 2) + i),
                        wqeff_sb[:],
                        start=True,
                        stop=True,
                    )
                pT_sb = small.tile([128, NJC // 2, H], BF16, tag=f"pT{g}")
                for gg in range(4):
                    gs = slice(gg * (NJC // 8), (gg + 1) * (NJC // 8))
                    nc.scalar.activation(
                        pT_sb[:, gs, :],
                        sT_ps[:, gs, :],
                        mybir.ActivationFunctionType.Exp,
                        scale=0.25,
                    )
                wctx.__exit__(None, None, None)
                pT_g.append(pT_sb)
                warm(NWARM3)
                u_ps = ps_small.tile([H, HP], F32, tag=f"u{g}")
                for i in range(NJC // 2):
                    nc.tensor.matmul(
                        u_ps[:],
                        pT_sb[:, i, :],
                        _hnp_chunk(hna_sb, g * (NJC // 2) + i),
                        start=(i == 0),
                        stop=(i == NJC // 2 - 1),
                    )
                u_g.append(u_ps)
                if g == 0:
                    # copy group-A partial to SBUF while group 2 streams in
                    # (on the idle vector engine: the scalar engine's
                    # in-order stream must not delay exp of group 2)
                    ua_sb = small.tile([H, HP], F32)
                    nc.vector.tensor_copy(ua_sb[:], u_ps[:])
                    warm(NWARM4)

            # u_sum = u_a(SBUF) + u_b(PSUM) -> SBUF (one PSUM operand only)
            usum_sb = small.tile([H, HP], F32)
            nc.vector.tensor_tensor(
                usum_sb[:], ua_sb[:], u_g[1][:], op=mybir.AluOpType.add
            )
            rs_sb = small.tile([H, 1], F32)
            nc.vector.reciprocal(rs_sb[:], usum_sb[:, HID : HID + 1])
            u_sb = small.tile([H, HID], BF16)
            nc.vector.tensor_scalar_mul(u_sb[:], usum_sb[:, 0:HID], rs_sb[:])

            uT_ps = ps_tmp.tile([HID, H], BF16, tag="tmp")
            nc.tensor.transpose(uT_ps[:], u_sb[:], id8[0:8, :])
            uT_sb = small.tile([HID, H], BF16)
            nc.scalar.copy(uT_sb[:], uT_ps[:])

            ym_ps = ps_tmp.tile([HID, H], F32, tag="tmp")
            nc.tensor.matmul(
                ym_ps[:], wb_sb[:, WV0 : WV0 + HID], uT_sb[:], start=True, stop=True
            )
            ymm_sb = small.tile([HID, H], F32)
            y_sb = small.tile([HID, 1], F32)
            nc.vector.tensor_mul(ymm_sb[:], ym_ps[:], mask_ap)
            nc.vector.tensor_reduce(
                y_sb[:], ymm_sb[:], axis=mybir.AxisListType.X, op=mybir.AluOpType.add
            )

            y01_sb = small.tile([HID, 2], BF16)
            nc.vector.tensor_scalar_mul(y01_sb[:], wl_rep, y_sb[:])
            mh_ps = ps_tmp.tile([HID, 2], F32, tag="tmp")
            nc.tensor.matmul(
                mh_ps[:],
                wb_sb[:, WMHC0 : WMHC0 + HID],
                y01_sb[:],
                start=True,
                stop=True,
            )
            mh1_sb = small.tile([HID, 1], BF16)
            nc.scalar.copy(mh1_sb[:], mh_ps[:, 1:2])

            # col[p, t] = W1*y2[t*128 + p] (rows are rotation-local); runs
            # on the PE while the vector engine broadcasts mh0
            col_ps = ps_small.tile([128, NT], F32)
            for t in range(NT):
                nc.tensor.matmul(
                    col_ps[:, t : t + 1],
                    _hnT_chunk(hna_sb, t),
                    mh1_sb[:],
                    start=True,
                    stop=True,
                )
            mh0rep_sb = small.tile([HID, HID], BF16)
            nc.vector.tensor_scalar_mul(mh0rep_sb[:], ones_sb[:], mh_ps[:, 0:1])
            nc.scalar.copy(col_sb[:], col_ps[:])

        # ---- R in PSUM (all 8 banks) -> SBUF copies chase the matmuls ----
        with tc.tile_pool(name="ps_r", bufs=1, space="PSUM") as ps_r:
            r_ps = ps_r.tile([128, 8, 512], F32)
            for k in range(8):
                nc.tensor.matmul(
                    r_ps[:, k, :],
                    mh0rep_sb[:],
                    _hnT_chunk(hna_sb, k, 512),
                    start=True,
                    stop=True,
                )
            r_flat = r_ps[:].rearrange("p a b -> p (a b)")

            # ---- epilogue: e tiles + DMA out (all on the sync ring) ----
            # tile 0 goes out in progressive pieces (512/512/1024/2048 cols)
            # so the first DMA fires right after R0; tiles 1..NSPLIT-1 go
            # half-width. All straight from PSUM and emitted BEFORE the
            # copies, so their only gate is the R matmuls (this also gives
            # the copies + pool-close DRAIN slack before the first
            # r_sb-reading tile).
            etile = epool.tile([128, N], F32)
            for c0, c1 in ((0, 512), (512, 1024), (1024, 2048), (2048, 4096)):
                nc.vector.tensor_scalar_add(
                    etile[:, c0:c1], r_flat[:, c0:c1], col_sb[:, 0:1]
                )
                nc.sync.dma_start(out_ext[0:128, c0:c1], etile[:, c0:c1])
            for t in range(1, NSPLIT):
                etile = epool.tile([128, N], F32)
                for hlf in range(2):
                    cs = slice(hlf * (N // 2), (hlf + 1) * (N // 2))
                    nc.vector.tensor_scalar_add(
                        etile[:, cs], r_flat[:, cs], col_sb[:, t : t + 1]
                    )
                    nc.sync.dma_start(out_ext[bass.ts(t, 128), cs], etile[:, cs])
            for k in range(8):
                nc.scalar.copy(r_sb[:, bass.ts(k, 512)], r_ps[:, k, :])
            for t in range(NSPLIT, NT):
                etile = epool.tile([128, N], F32)
                nc.vector.tensor_scalar_add(
                    etile[:], r_sb[:], col_sb[:, t : t + 1]
                )
                nc.sync.dma_start(out_ext[bass.ts(t, 128), :], etile[:])

    nc.finalize()
    return nc


_CACHED = {}


def _get_nc():
    if "nc" not in _CACHED:
        _CACHED["nc"] = build_bass()
    return _CACHED["nc"]


def _make_wb(W_q, W_kv, W_mhc, W_lin):
    wb = np.zeros((HID, WB_COLS), dtype=ml_dtypes.bfloat16)
    wb[:, WKT0 : WKT0 + HID] = W_kv[:, :HID].T
    wb[:, WV0 : WV0 + HID] = W_kv[:, HID:]
    wb[:, WMHC0 : WMHC0 + HID] = W_mhc
    wb[:, WQ0 : WQ0 + HID] = W_q
    wb[0:8, ID0 : ID0 + 8] = np.eye(8, dtype=np.float32)
    for hh in range(H):
        wb[hh * D : (hh + 1) * D, MSK0 + hh] = 1.0
    wb[:, WL0] = W_lin[0, 0]
    wb[:, WL0 + 1] = W_lin[1, 0]
    return wb


def kernel(h, W_q, W_kv, W_mhc, W_lin, _trace=False):
    h = np.ascontiguousarray(np.asarray(h, dtype=np.float32))
    W_q = np.asarray(W_q, dtype=np.float32)
    W_kv = np.asarray(W_kv, dtype=np.float32)
    W_mhc = np.asarray(W_mhc, dtype=np.float32)
    W_lin = np.asarray(W_lin, dtype=np.float32)

    nc = _get_nc()
    wb0 = _make_wb(W_q, W_kv, W_mhc, W_lin)

    in_maps = []
    for core in range(8):
        b, half = core // 2, core % 2
        wb = wb0.copy()
        wb[:, HGB0] = h[b, N, :].astype(ml_dtypes.bfloat16)
        # rotate the node axis so this core's own rows are chunks 0..15;
        # the output columns are un-rotated on the host below.
        hnb = np.roll(h[b, :N, :].astype(ml_dtypes.bfloat16), -half * ROWS, axis=0)
        hn3 = hnb.reshape(NJC, 128, HID).transpose(1, 0, 2)
        hnp = np.concatenate(
            [hn3, np.ones((128, NJC, 1), dtype=ml_dtypes.bfloat16)], axis=2
        )
        hnT = np.ascontiguousarray(hnb.T)
        hna = np.concatenate(
            [
                hnT[:, : N // 2],
                hnp[:, 0:16].reshape(128, 16 * HP),
                hnT[:, N // 2 :],
                hnp[:, 16:32].reshape(128, 16 * HP),
            ],
            axis=1,
        )
        in_maps.append({"wb": wb, "hna": np.ascontiguousarray(hna)})

    import time as _time

    kw = {}
    if _trace:
        import os

        kw = {"tmpdir": "/tmp/ktrace_" + str(os.getpid())}
        os.makedirs(kw["tmpdir"], exist_ok=True)
        print("[kernel] trace dir:", kw["tmpdir"], flush=True)
    _t = _time.time()
    print("[kernel] launching run_bass_kernel_spmd", flush=True)
    res = run_bass_kernel_spmd(nc, in_maps, core_ids=list(range(8)), trace=_trace, **kw)
    print(f"[kernel] run_bass_kernel_spmd done in {_time.time()-_t:.1f}s", flush=True)

    out = np.empty((BP, N * N, 1), dtype=np.float32)
    for core in range(8):
        b, half = core // 2, core % 2
        blk = res.results[core]["out"]  # (2048, 4096), columns rotated
        blk = np.roll(blk, half * ROWS, axis=1) if half else blk
        out[b, half * ROWS * N : (half + 1) * ROWS * N, 0] = blk.ravel()
    if _trace:
        return out, res
    return out


# revision 37
# speedup vs baseline: 1.1629x; 1.1629x over previous
"""Trainium2 Bass kernel for nn_AttentionEdgeDecoder.

Reference computation (per batch b):
  hn = h[b,:4096,:], hg = h[b,4096,:]
  q = hg @ W_q  (single query, 8 heads x 16 dims)
  k,v = hn @ W_kv ; attn = softmax(q.k/sqrt(16)) ; y = attn.v
  mh = y @ W_mhc ; y2[i] = <mh, hn[i]>             (4096 scalars)
  e[i,j] = y2[j]*W_lin[0,0] + y2[i]*W_lin[1,0]     (4096x4096 output)

Output is 4*4096^2*4B = 268MB -> HBM-write bound. Per-core the 33.5MB
write stream runs at ~427 GB/s (the DMA cap) and takes ~78us, so total
time = (time to first output byte) + 78us: the whole game is collapsing
the prologue. Sharding: 8 cores = 4 batches x 2 row-halves.

Trace-derived hardware facts this layout is built around:
 - HWDGE descriptor cost is ~fixed per descriptor, so DMA throughput is
   proportional to the per-partition line size; and the DMA engines drain
   the two rings in an unpredictable order when both have queued work, so
   ALL inputs go on the scalar ring in strict FIFO order (~305 GB/s with
   8.2KB lines) and the sync ring is output-only. hnT and hnp are shipped
   as ONE tensor in two column groups so the group-1 attention chain runs
   while group 2 is still streaming (score/exp/u tiles are split per
   group because dependency tracking is per-tile for multi-writer tiles,
   and a tile_wait_until hint stops the scheduler from hoisting the
   group-2 scores ahead of group-1's u matmuls).
 - DVE reading PSUM is ~1.6x slower than SBUF (1.1 vs 0.69 ns/elem), so
   most of the 16-tile e stream reads R from SBUF; the first NSPLIT
   row-tiles (latency-critical) read R straight from PSUM while the
   copies + PSUM-pool-close DRAIN finish in their shadow.
 - The PE p-state ramp needs ~3-4us of continuous high-utilization work;
   a small burst of bf16 warm-up matmuls on a memset tile (no DRAM
   dependency) overlaps the preamble. Mid-stream keep-hot bursts do NOT
   work: the tile scheduler hoists independent matmuls ahead of
   data-gated ones, delaying the real pipeline.

Per-core row ownership is made SPMD-uniform by ROTATING each core's node
axis so its own 2048 rows come first (host rolls hn by half*2048; the
host un-rotates the output columns after the run). This removes the hrT
input entirely: the col term's lhsT tiles are just hnT[:, t*128:...].

TensorEngine formulation (out = lhsT.T @ rhs), heavy ops bf16 1-pass:
  q_col   = matmul(lhsT=W_q, rhs=hg_col)                  [128,1]
  Qh      = headmask * q_col   (block-diag scatter)       [128,8]
  Wqeff   = matmul(lhsT=WkT, rhs=Qh) = Wk @ Qh            [128,8]
  sT      = matmul(lhsT=hnT_chunk, rhs=Wqeff)             [4096,8]
  pT      = exp(0.25*sT)      (no max-subtract: |s/4| < 8)
  u_ext   = sum_chunks matmul(lhsT=pT_chunk, rhs=hnp_chunk) [8,129]
            (hnp carries a ones column -> u_ext[:,128] = softmax denom)
  u       = u_ext[:, :128] * (1/ssum)  (one vector op PSUM->SBUF)
  uT      = transpose(u)                                  [128,8]
  ymatT   = matmul(lhsT=Wv, rhs=uT)                       [128,8]
  y_col   = reduce_h(ymatT * headmask)                    [128,1]
  y01     = y_col * [W0, W1]  (one vector op)             [128,2]
  mh01    = matmul(lhsT=W_mhc, rhs=y01)                   [128,2]
            -> mh0 (f32 col), mh1 (bf16 col); no K=1 broadcast matmuls
  mh0rep  = ones * mh0  (vector bcast along free, bf16)   [128,128]
  col     = matmul(lhsT=hnT_tile_t, rhs=mh1) = W1*y2[rows] [128,16]
  R       = matmul(lhsT=mh0rep, rhs=hnT) in PSUM          [128,4096]
            -> copied to SBUF r_sb chunk-by-chunk (scalar engine)
  e_tile  = tensor_scalar_add(R, col[:,t]) -> 2MB DMAs out (sync ring)

Tile 0 streams out in progressive pieces (512/512/1024/2048 cols) so
the first DMA fires right after R's first bank; group 2 of the input is
two DMAs (hnT then hnp) so the group-2 scores start before its hnp half
has landed.

Best measured: 110.4us (baseline at session start: 123.8us). Note the
device is bimodally noisy run-to-run (~110 vs ~129 for identical NEFFs,
apparently HBM contention external to this kernel).
"""

from contextlib import ExitStack

import ml_dtypes
import numpy as np

import concourse.bass as bass
import concourse.mybir as mybir
from concourse import bacc, tile
from concourse.bass_utils import run_bass_kernel_spmd

BP = 4
N = 4096
HID = 128
H = 8
D = 16
ROWS = N // 2          # 2048 rows per core
NT = ROWS // 128       # 16 row tiles per core
NJC = N // 128         # 32 node chunks
HP = HID + 1           # hnp chunk width (extra ones column -> softmax denom)
F32 = mybir.dt.float32
BF16 = mybir.dt.bfloat16

NWARM = 6              # PE warm-up matmuls (bf16 [128,512]) for p-state ramp
NWARM2 = 0             # (the tile scheduler hoists independent warm matmuls
NWARM3 = 0             #  ahead of data-gated work, so mid-stream keep-hot
NWARM4 = 0             #  bursts only delay the real pipeline; keep them off)
NSPLIT = 4             # first NSPLIT row tiles stream as half-width DMAs

# hna column groups: [hnT chunks 0-15 | hnp chunks 0-15 | hnT 16-31 | hnp 16-31]
# so that scores/u for the first 16 node chunks can start when the first
# group-pair of DMAs lands, pipelining compute against the input stream.
G1T = 0
G1P = N // 2                   # 2048
G2T = G1P + 16 * HP            # 4112
G2P = G2T + N // 2             # 6160
HNA_COLS = G2P + 16 * HP       # 8224


def _hnT_chunk(hna, jc, w=128):
    """AP of hnT columns [jc*w, (jc+1)*w) in the grouped layout (w | 2048)."""
    j0 = jc * w
    base = j0 if j0 < N // 2 else G2T + (j0 - N // 2)
    return hna[:, base : base + w]


def _hnp_chunk(hna, jc):
    base = (G1P + jc * HP) if jc < 16 else (G2P + (jc - 16) * HP)
    return hna[:, base : base + HP]

# wb column layout (bf16)
WKT0 = 0               # WkT = W_kv[:, :128].T
WV0 = HID              # Wv  = W_kv[:, 128:]
WMHC0 = 2 * HID
WQ0 = 3 * HID
HGB0 = 4 * HID         # hg column
ID0 = HGB0 + 1         # 8x8 identity (partitions 0..7)
MSK0 = ID0 + 8         # head mask [128, 8]
WL0 = MSK0 + 8         # wl_rep[:, k] = W_lin[k, 0] on every partition
WB_COLS = WL0 + 2


def build_bass():
    nc = bacc.Bacc()

    wb_ext = nc.declare_dram_parameter("wb", [HID, WB_COLS], BF16, isOutput=False)
    hna_ext = nc.declare_dram_parameter("hna", [128, HNA_COLS], BF16, isOutput=False)
    out_ext = nc.declare_dram_parameter("out", [ROWS, N], F32, isOutput=True)

    with tile.TileContext(nc) as tc, ExitStack() as ctx:
        sb = ctx.enter_context(tc.tile_pool(name="sb", bufs=1))
        small = ctx.enter_context(tc.tile_pool(name="small", bufs=1))
        epool = ctx.enter_context(tc.tile_pool(name="epool", bufs=8))

        # ---- input DMAs: 8.2KB lines, split across rings by partition and
        # into two column groups so compute pipelines against arrival.
        wb_sb = sb.tile([HID, WB_COLS], BF16)
        hna_sb = sb.tile([128, HNA_COLS], BF16)
        # all inputs on the scalar ring in strict FIFO order (the DMA
        # engines drain rings in an unpredictable order when both have
        # queued work, so a single ring is the only way to guarantee
        # group 1 lands before group 2); the sync ring is output-only.
        nc.scalar.dma_start(wb_sb[:], wb_ext[:, :])
        nc.scalar.dma_start(hna_sb[:, :G2T], hna_ext[:, :G2T])
        nc.scalar.dma_start(hna_sb[:, G2T:G2P], hna_ext[:, G2T:G2P])
        nc.scalar.dma_start(hna_sb[:, G2P:], hna_ext[:, G2P:])

        id8 = wb_sb[:, ID0 : ID0 + 8]
        mask_ap = wb_sb[:, MSK0 : MSK0 + H]
        wl_rep = wb_sb[:, WL0 : WL0 + 2]

        r_sb = sb.tile([128, N], F32)
        col_sb = small.tile([128, NT], F32)

        with ExitStack() as pctx:
            ps_small = pctx.enter_context(
                tc.tile_pool(name="ps_small", bufs=1, space="PSUM")
            )
            ps_tmp = pctx.enter_context(
                tc.tile_pool(name="ps_tmp", bufs=2, space="PSUM")
            )
            ps_warm = pctx.enter_context(
                tc.tile_pool(name="ps_warm", bufs=1, space="PSUM")
            )

            # ---- PE warm-up: no DRAM dependency, ramps the p-state clock
            warm_sb = small.tile([128, 512], BF16)
            nc.vector.memset(warm_sb[:], 1.0)
            ones_sb = small.tile([128, HID], BF16)
            nc.vector.memset(ones_sb[:], 1.0)
            for _ in range(NWARM):
                warm_ps = ps_warm.tile([128, 512], F32, tag="warm")
                nc.tensor.matmul(
                    warm_ps[:], warm_sb[:, :128], warm_sb[:], start=True, stop=True
                )

            # ---- attention prologue ----
            q_ps = ps_tmp.tile([HID, 1], F32, tag="tmp")
            nc.tensor.matmul(
                q_ps[:],
                wb_sb[:, WQ0 : WQ0 + HID],
                wb_sb[:, HGB0 : HGB0 + 1],
                start=True,
                stop=True,
            )
            q_sb = small.tile([HID, 1], F32)
            nc.scalar.copy(q_sb[:], q_ps[:])

            qh_sb = small.tile([HID, H], BF16)
            nc.vector.tensor_scalar_mul(qh_sb[:], mask_ap, q_sb[:])

            wq_ps = ps_tmp.tile([HID, H], F32, tag="tmp")
            nc.tensor.matmul(
                wq_ps[:], wb_sb[:, WKT0 : WKT0 + HID], qh_sb[:], start=True, stop=True
            )
            wqeff_sb = small.tile([HID, H], BF16)
            nc.scalar.copy(wqeff_sb[:], wq_ps[:])

            def warm(n):
                for _ in range(n):
                    wps = ps_warm.tile([128, 512], F32, tag="warm")
                    nc.tensor.matmul(
                        wps[:], warm_sb[:, :128], warm_sb[:], start=True, stop=True
                    )

            # keep the PE busy until input group 1 lands
            warm(NWARM2)

            # scores + exp + u per input column-group (separate tiles so the
            # group-1 chain runs while group 2 is still streaming in; dep
            # tracking is per-tile for multi-writer tiles). Keep-hot warm
            # matmuls fill the PE gaps so the p-state clock stays up.
            pT_g = []
            u_g = []
            for g in range(2):
                # tell the scheduler group-2 data really lands late, so it
                # orders group-1's u matmuls ahead of the group-2 scores
                wctx = tc.tile_wait_until(0.016, enable=(g == 1))
                wctx.__enter__()
                sT_ps = ps_small.tile([128, NJC // 2, H], F32, tag=f"sT{g}")
                for i in range(NJC // 2):
                    nc.tensor.matmul(
                        sT_ps[:, i, :],
                        _hnT_chunk(hna_sb, g * (NJC // 2) + i),
                        wqeff_sb[:],
                        start=True,
                        stop=True,
                    )
                pT_sb = small.tile([128, NJC // 2, H], BF16, tag=f"pT{g}")
                for gg in range(4):
                    gs = slice(gg * (NJC // 8), (gg + 1) * (NJC // 8))
                    nc.scalar.activation(
                        pT_sb[:, gs, :],
                        sT_ps[:, gs, :],
                        mybir.ActivationFunctionType.Exp,
                        scale=0.25,
                    )
                wctx.__exit__(None, None, None)
                pT_g.append(pT_sb)
                warm(NWARM3)
                u_ps = ps_small.tile([H, HP], F32, tag=f"u{g}")
                for i in range(NJC // 2):
                    nc.tensor.matmul(
                        u_ps[:],
                        pT_sb[:, i, :],
                        _hnp_chunk(hna_sb, g * (NJC // 2) + i),
                        start=(i == 0),
                        stop=(i == NJC // 2 - 1),
                    )
                u_g.append(u_ps)
                if g == 0:
                    # copy group-A partial to SBUF while group 2 streams in
                    # (on the idle vector engine: the scalar engine's
                    # in-order stream must not delay exp of group 2)
                    ua_sb = small.tile([H, HP], F32)
                    nc.vector.tensor_copy(ua_sb[:], u_ps[:])
                    warm(NWARM4)

            # u_sum = u_a(SBUF) + u_b(PSUM) -> SBUF (one PSUM operand only)
            usum_sb = small.tile([H, HP], F32)
            nc.vector.tensor_tensor(
                usum_sb[:], ua_sb[:], u_g[1][:], op=mybir.AluOpType.add
            )
            rs_sb = small.tile([H, 1], F32)
            nc.vector.reciprocal(rs_sb[:], usum_sb[:, HID : HID + 1])
            u_sb = small.tile([H, HID], BF16)
            nc.vector.tensor_scalar_mul(u_sb[:], usum_sb[:, 0:HID], rs_sb[:])

            uT_ps = ps_tmp.tile([HID, H], BF16, tag="tmp")
            nc.tensor.transpose(uT_ps[:], u_sb[:], id8[0:8, :])
            uT_sb = small.tile([HID, H], BF16)
            nc.scalar.copy(uT_sb[:], uT_ps[:])

            ym_ps = ps_tmp.tile([HID, H], F32, tag="tmp")
            nc.tensor.matmul(
                ym_ps[:], wb_sb[:, WV0 : WV0 + HID], uT_sb[:], start=True, stop=True
            )
            ymm_sb = small.tile([HID, H], F32)
            y_sb = small.tile([HID, 1], F32)
            nc.vector.tensor_mul(ymm_sb[:], ym_ps[:], mask_ap)
            nc.vector.tensor_reduce(
                y_sb[:], ymm_sb[:], axis=mybir.AxisListType.X, op=mybir.AluOpType.add
            )

            y01_sb = small.tile([HID, 2], BF16)
            nc.vector.tensor_scalar_mul(y01_sb[:], wl_rep, y_sb[:])
            mh_ps = ps_tmp.tile([HID, 2], F32, tag="tmp")
            nc.tensor.matmul(
                mh_ps[:],
                wb_sb[:, WMHC0 : WMHC0 + HID],
                y01_sb[:],
                start=True,
                stop=True,
            )
            mh1_sb = small.tile([HID, 1], BF16)
            nc.scalar.copy(mh1_sb[:], mh_ps[:, 1:2])

            # col[p, t] = W1*y2[t*128 + p] (rows are rotation-local); runs
            # on the PE while the vector engine broadcasts mh0
            col_ps = ps_small.tile([128, NT], F32)
            for t in range(NT):
                nc.tensor.matmul(
                    col_ps[:, t : t + 1],
                    _hnT_chunk(hna_sb, t),
                    mh1_sb[:],
                    start=True,
                    stop=True,
                )
            mh0rep_sb = small.tile([HID, HID], BF16)
            nc.vector.tensor_scalar_mul(mh0rep_sb[:], ones_sb[:], mh_ps[:, 0:1])
            nc.scalar.copy(col_sb[:], col_ps[:])

        # ---- R in PSUM (all 8 banks) -> SBUF copies chase the matmuls ----
        with tc.tile_pool(name="ps_r", bufs=1, space="PSUM") as ps_r:
            r_ps = ps_r.tile([128, 8, 512], F32)
            for k in range(8):
                nc.tensor.matmul(
                    r_ps[:, k, :],
                    mh0rep_sb[:],
                    _hnT_chunk(hna_sb, k, 512),
                    start=True,
                    stop=True,
                )
            r_flat = r_ps[:].rearrange("p a b -> p (a b)")

            # ---- epilogue: e tiles + DMA out (all on the sync ring) ----
            # tile 0 goes out in progressive pieces (512/512/1024/2048 cols)
            # so the first DMA fires right after R0; tiles 1..NSPLIT-1 go
            # half-width. All straight from PSUM and emitted BEFORE the
            # copies, so their only gate is the R matmuls (this also gives
            # the copies + pool-close DRAIN slack before the first
            # r_sb-reading tile).
            etile = epool.tile([128, N], F32)
            # piece 1 on the scalar engine (Identity activation with a
            # per-partition bias AP computes in+col), in parallel with the
            # vector engine's piece 2
            nc.scalar.activation(
                etile[:, 0:512],
                r_flat[:, 0:512],
                mybir.ActivationFunctionType.Identity,
                bias=col_sb[:, 0:1],
            )
            nc.sync.dma_start(out_ext[0:128, 0:512], etile[:, 0:512])
            for c0, c1 in ((512, 1024), (1024, 2048), (2048, 4096)):
                nc.vector.tensor_scalar_add(
                    etile[:, c0:c1], r_flat[:, c0:c1], col_sb[:, 0:1]
                )
                nc.sync.dma_start(out_ext[0:128, c0:c1], etile[:, c0:c1])
            for t in range(1, NSPLIT):
                etile = epool.tile([128, N], F32)
                for hlf in range(2):
                    cs = slice(hlf * (N // 2), (hlf + 1) * (N // 2))
                    nc.vector.tensor_scalar_add(
                        etile[:, cs], r_flat[:, cs], col_sb[:, t : t + 1]
                    )
                    nc.sync.dma_start(out_ext[bass.ts(t, 128), cs], etile[:, cs])
            for k in range(8):
                nc.scalar.copy(r_sb[:, bass.ts(k, 512)], r_ps[:, k, :])
            for t in range(NSPLIT, NT):
                etile = epool.tile([128, N], F32)
                nc.vector.tensor_scalar_add(
                    etile[:], r_sb[:], col_sb[:, t : t + 1]
                )
                nc.sync.dma_start(out_ext[bass.ts(t, 128), :], etile[:])

    nc.finalize()
    return nc


_CACHED = {}


def _get_nc():
    if "nc" not in _CACHED:
        _CACHED["nc"] = build_bass()
    return _CACHED["nc"]


def _make_wb(W_q, W_kv, W_mhc, W_lin):
    wb = np.zeros((HID, WB_COLS), dtype=ml_dtypes.bfloat16)
    wb[:, WKT0 : WKT0 + HID] = W_kv[:, :HID].T
    wb[:, WV0 : WV0 + HID] = W_kv[:, HID:]
    wb[:, WMHC0 : WMHC0 + HID] = W_mhc
    wb[:, WQ0 : WQ0 + HID] = W_q
    wb[0:8, ID0 : ID0 + 8] = np.eye(8, dtype=np.float32)
    for hh in range(H):
        wb[hh * D : (hh + 1) * D, MSK0 + hh] = 1.0
    wb[:, WL0] = W_lin[0, 0]
    wb[:, WL0 + 1] = W_lin[1, 0]
    return wb


def kernel(h, W_q, W_kv, W_mhc, W_lin, _trace=False):
    h = np.ascontiguousarray(np.asarray(h, dtype=np.float32))
    W_q = np.asarray(W_q, dtype=np.float32)
    W_kv = np.asarray(W_kv, dtype=np.float32)
    W_mhc = np.asarray(W_mhc, dtype=np.float32)
    W_lin = np.asarray(W_lin, dtype=np.float32)

    nc = _get_nc()
    wb0 = _make_wb(W_q, W_kv, W_mhc, W_lin)

    in_maps = []
    for core in range(8):
        b, half = core // 2, core % 2
        wb = wb0.copy()
        wb[:, HGB0] = h[b, N, :].astype(ml_dtypes.bfloat16)
        # rotate the node axis so this core's own rows are chunks 0..15;
        # the output columns are un-rotated on the host below.
        hnb = np.roll(h[b, :N, :].astype(ml_dtypes.bfloat16), -half * ROWS, axis=0)
        hn3 = hnb.reshape(NJC, 128, HID).transpose(1, 0, 2)
        hnp = np.concatenate(
            [hn3, np.ones((128, NJC, 1), dtype=ml_dtypes.bfloat16)], axis=2
        )
        hnT = np.ascontiguousarray(hnb.T)
        hna = np.concatenate(
            [
                hnT[:, : N // 2],
                hnp[:, 0:16].reshape(128, 16 * HP),
                hnT[:, N // 2 :],
                hnp[:, 16:32].reshape(128, 16 * HP),
            ],
            axis=1,
        )
        in_maps.append({"wb": wb, "hna": np.ascontiguousarray(hna)})

    import time as _time

    kw = {}
    if _trace:
        import os

        kw = {"tmpdir": "/tmp/ktrace_" + str(os.getpid())}
        os.makedirs(kw["tmpdir"], exist_ok=True)
        print("[kernel] trace dir:", kw["tmpdir"], flush=True)
    _t = _time.time()
    print("[kernel] launching run_bass_kernel_spmd", flush=True)
    res = run_bass_kernel_spmd(nc, in_maps, core_ids=list(range(8)), trace=_trace, **kw)
    print(f"[kernel] run_bass_kernel_spmd done in {_time.time()-_t:.1f}s", flush=True)

    out = np.empty((BP, N * N, 1), dtype=np.float32)
    for core in range(8):
        b, half = core // 2, core % 2
        blk = res.results[core]["out"]  # (2048, 4096), columns rotated
        blk = np.roll(blk, half * ROWS, axis=1) if half else blk
        out[b, half * ROWS * N : (half + 1) * ROWS * N, 0] = blk.ravel()
    if _trace:
        return out, res
    return out
